# revision 18
# baseline (speedup 1.0000x reference)
"""Trainium2 Bass kernel for 16-head MultiHeadAttention.

Problem shapes (hardcoded): B=2, L=2048, D=1024, H=16, DK=64, fp32.

Sharding over 8 cores: core c handles batch b=c//4 and head-group g=c%4
(4 heads, 256 of the 1024 QKV columns).  Per core:

  XT = X^T via PE transposes                  [1024, 2048]
  QT = WQc^T X^T + bQc   (c-rows, l-cols)     [256, 2048]
  KT = WKc^T X^T + bKc                        [256, 2048]
  Vaug = X WVaug + bVaug  (V cols per head followed by a ones column,
         produced by zero-cols in WVaug and 1.0 entries in bVaug)
  per head h, per l(=q) chunk:
      S^T tile  = KT_h^T-slice x QT_h          (scores, [k,q] layout)
      U = exp(S^T / 8)                          (ACT, scale fused)
      OT[65, q] += [V_h | 1]^T @ U              (row 64 = softmax denom)
      OTn = OT[0:64] * recip(bcast(OT[64]))     (DMA partition-broadcast)
  AllToAll over all 8 cores exchanges OTn l-slices so that core c ends up
  with all 1024 c-rows (16 heads) for l-rows {128c..128c+127} of BOTH
  halves of the sequence, for its batch -- and, from the other batch
  group, the same rows of the other batch.  Then
      Y[128, 1024] = OTn_all^T @ WO + bO        (full WO, no reduction)

Output per core: [2 batches, 2 halves, 128, 1024]; host concatenates.
"""

import numpy as np
from contextlib import ExitStack

import concourse.bass as bass
import concourse.bacc as bacc
import concourse.mybir as mybir
import concourse.tile as tile
from concourse.bass_utils import run_bass_kernel_spmd
from concourse.masks import make_identity

F32 = mybir.dt.float32
F32R = mybir.dt.float32r
BF16 = mybir.dt.bfloat16
AF = mybir.ActivationFunctionType


def R(ap):
    """View an fp32 AP as float32r for full-rate PE matmuls."""
    return ap.bitcast(F32R)

B, L, D, H, DK = 2, 2048, 1024, 16, 64
NCORES = 8
NH = 4            # heads per core
CPC = NH * DK     # 256 qkv cols per core
VA = NH * (DK + 1)  # 260, V-aug width (per-head [V | 1])
LT = L // 128     # 16 l-chunks
DCH = D // 128    # 8 d-chunks


def _emit(tc, nc, x, wq, bq, wk, bk, wv, bv, wo, bo, out):
    with ExitStack() as es:
        # ---------------- persistent pools ----------------
        const = es.enter_context(tc.tile_pool(name="const", bufs=1))
        ident_f32 = const.tile([128, 128], F32)
        make_identity(nc, ident_f32)
        ident = const.tile([128, 128], F32R)
        nc.vector.tensor_copy(ident, ident_f32)

        wq_sb = const.tile([128, DCH, CPC], F32R)
        nc.sync.dma_start(wq_sb, wq.ap().rearrange("(dc p) c -> p dc c", p=128))
        wk_sb = const.tile([128, DCH, CPC], F32R)
        nc.sync.dma_start(wk_sb, wk.ap().rearrange("(dc p) c -> p dc c", p=128))
        wv_sb = const.tile([128, DCH, VA], F32R)
        nc.sync.dma_start(wv_sb, wv.ap().rearrange("(dc p) c -> p dc c", p=128))

        bq_sb = const.tile([128, 2, 1], F32)
        nc.sync.dma_start(bq_sb, bq.ap().rearrange("(m p) o -> p m o", p=128))
        bk_sb = const.tile([128, 2, 1], F32)
        nc.sync.dma_start(bk_sb, bk.ap().rearrange("(m p) o -> p m o", p=128))

        bv_bc = const.tile([128, VA], F32)
        bv_ap = bv.ap()
        nc.gpsimd.dma_start(
            bv_bc, bass.AP(tensor=bv_ap.tensor, offset=bv_ap.offset,
                           ap=[[0, 128]] + list(bv_ap.ap[1:]))
        )
        bo_bc = const.tile([128, D], F32)
        bo_ap = bo.ap()
        nc.gpsimd.dma_start(
            bo_bc, bass.AP(tensor=bo_ap.tensor, offset=bo_ap.offset,
                           ap=[[0, 128]] + list(bo_ap.ap[1:]))
        )

        proj = es.enter_context(tc.tile_pool(name="proj", bufs=1))
        qt = proj.tile([128, 2, L], BF16)       # Q^T, c-chunk m rows
        kt = proj.tile([128, 2, L], BF16)       # K^T
        vaug = proj.tile([128, LT, VA], BF16)   # [V | 1] per l-chunk
        otn4 = proj.tile([64, NH, L], F32R)     # normalized O^T per head

        # ---------------- phase 1+2: XT + projections ----------------
        with ExitStack() as ph1:
            xtp = ph1.enter_context(tc.tile_pool(name="xtp", bufs=1))
            xt = xtp.tile([128, DCH, L], F32R)  # X^T (d on partitions)
            xload = ph1.enter_context(tc.tile_pool(name="xload", bufs=2))
            pst = ph1.enter_context(tc.tile_pool(name="pst", bufs=2, space="PSUM"))
            ppq = ph1.enter_context(tc.tile_pool(name="ppq", bufs=2, space="PSUM"))
            ppv = ph1.enter_context(tc.tile_pool(name="ppv", bufs=2, space="PSUM"))

            xap = x.ap()
            for lt in range(LT):
                xrow = xload.tile([128, D], F32R, tag="xrow")
                nc.sync.dma_start(xrow, xap[lt * 128:(lt + 1) * 128, :])
                for dg in range(2):
                    pt = pst.tile([128, 512], F32R, tag="pt")
                    for q in range(4):
                        dc = dg * 4 + q
                        nc.tensor.transpose(
                            pt[:, q * 128:(q + 1) * 128],
                            xrow[:, dc * 128:(dc + 1) * 128], ident)
                    nc.any.tensor_copy(
                        xt[:, dg * 4:dg * 4 + 4, lt * 128:(lt + 1) * 128],
                        pt.rearrange("p (a b) -> p a b", a=4))

            # K^T and Q^T projections, per c-chunk m (order: m=0 first so
            # heads 0/1 attention can start early)
            for m in range(2):
                for w_sb, b_sb, dst in ((wk_sb, bk_sb, kt), (wq_sb, bq_sb, qt)):
                    for nq in range(2):
                        pq = ppq.tile([128, 1024], F32, tag="pq")
                        for dc in range(DCH):
                            for nn in range(2):
                                nc.tensor.matmul(
                                    pq[:, nn * 512:(nn + 1) * 512],
                                    R(w_sb[:, dc, m * 128:(m + 1) * 128]),
                                    R(xt[:, dc, nq * 1024 + nn * 512:
                                         nq * 1024 + (nn + 1) * 512]),
                                    start=(dc == 0), stop=(dc == DCH - 1))
                        nc.vector.tensor_scalar_add(
                            dst[:, m, nq * 1024:(nq + 1) * 1024], pq,
                            b_sb[:, m, :])
                # V projection rows for this half of l handled below (needs
                # all d-chunks of xt as lhsT) -- emitted once, after m=0.
                if m == 0:
                    for lt in range(LT):
                        pv = ppv.tile([128, VA], F32, tag="pv")
                        for dc in range(DCH):
                            nc.tensor.matmul(
                                pv, R(xt[:, dc, lt * 128:(lt + 1) * 128]),
                                R(wv_sb[:, dc, :]),
                                start=(dc == 0), stop=(dc == DCH - 1))
                        nc.vector.tensor_add(vaug[:, lt, :], pv, bv_bc)

        # ---------------- phase 3: attention + exchange + output ----------
        with ExitStack() as ph3:
            wop = ph3.enter_context(tc.tile_pool(name="wop", bufs=1))
            wo_sb = wop.tile([128, DCH, D], F32R)
            nc.sync.dma_start(wo_sb, wo.ap().rearrange("(j p) n -> p j n", p=128))

            psA = ph3.enter_context(tc.tile_pool(name="psA", bufs=2, space="PSUM"))
            psB = ph3.enter_context(tc.tile_pool(name="psB", bufs=1, space="PSUM"))
            psY = ph3.enter_context(tc.tile_pool(name="psY", bufs=1, space="PSUM"))
            upool = ph3.enter_context(tc.tile_pool(name="upool", bufs=4))
            npool = ph3.enter_context(tc.tile_pool(name="npool", bufs=2))
            opool = ph3.enter_context(tc.tile_pool(name="opool", bufs=2))
            ypool = ph3.enter_context(tc.tile_pool(name="ypool", bufs=2))
            dram = ph3.enter_context(tc.tile_pool(name="dram", bufs=2, space="DRAM"))

            outap = out.ap()

            def emit_attention(half):
                q0 = half * 1024
                for h in range(NH):
                    m, r0 = h // 2, (h % 2) * 64
                    otp = psB.tile([65, 1024], F32, tag="otp")
                    for kti in range(LT):
                        sp = psA.tile([128, 1024], F32, tag="sp")
                        for nn in range(2):
                            nc.tensor.matmul(
                                sp[:, nn * 512:(nn + 1) * 512],
                                kt[r0:r0 + 64, m, kti * 128:(kti + 1) * 128],
                                qt[r0:r0 + 64, m,
                                   q0 + nn * 512:q0 + (nn + 1) * 512],
                                start=True, stop=True)
                        u = upool.tile([128, 1024], BF16, tag="u")
                        nc.scalar.activation(u, sp, AF.Exp, scale=0.125)
                        for nn in range(2):
                            nc.tensor.matmul(
                                otp[:, nn * 512:(nn + 1) * 512],
                                vaug[:, kti, h * 65:(h + 1) * 65],
                                u[:, nn * 512:(nn + 1) * 512],
                                start=(kti == 0), stop=(kti == LT - 1))
                    # evacuate PSUM immediately so the next head's AV can
                    # start; the normalization chain then runs off-path.
                    otu = npool.tile([65, 1024], F32, tag="otu")
                    nc.vector.tensor_copy(otu, otp)
                    sdr = dram.tile([1, 1024], F32, tag="sdr")
                    nc.sync.dma_start(sdr, otu[64:65, :])
                    sbc = npool.tile([64, 1024], F32, tag="sbc")
                    sdr_ap = sdr
                    nc.sync.dma_start(
                        sbc, bass.AP(tensor=sdr_ap.tensor, offset=sdr_ap.offset,
                                     ap=[[0, 64]] + list(sdr_ap.ap[1:])))
                    rec = npool.tile([64, 1024], F32, tag="rec")
                    nc.vector.reciprocal_approx_fast(rec, sbc)
                    nc.vector.tensor_mul(
                        otn4[:, h, q0:q0 + 1024], otu[0:64, :], rec)

            def emit_exchange(half):
                # ship this half: chunk s -> core s (l-rows 128s of the half)
                q0 = half * 1024
                ain = dram.tile([8, CPC, 128], F32R, tag="ain",
                                name=f"ain{half}")
                for h in range(NH):
                    nc.sync.dma_start(
                        ain[:, h * 64:(h + 1) * 64, :].rearrange(
                            "s p l -> p s l"),
                        otn4[:, h, q0:q0 + 1024].rearrange(
                            "p (s l) -> p s l", s=8))
                aout = dram.tile([8, CPC, 128], F32R, tag="aout",
                                 name=f"aout{half}")
                nc.gpsimd.collective_compute(
                    "AllToAll", mybir.AluOpType.bypass,
                    replica_groups=[list(range(NCORES))],
                    ins=[ain.opt()], outs=[aout.opt()])
                return aout

            def emit_output(half, aout):
                for b in range(B):
                    oall = opool.tile([128, DCH, 128], F32R, tag="oall",
                                      name=f"oall{half}{b}")
                    for s in range(4):
                        nc.sync.dma_start(
                            oall[:, 2 * s:2 * s + 2, :],
                            aout[4 * b + s, :, :].rearrange(
                                "(t p) l -> p t l", p=128))
                    y_ps = psY.tile([128, 1024], F32, tag="yps",
                                    name=f"yps{half}{b}")
                    for j in range(DCH):
                        for nn in range(2):
                            nc.tensor.matmul(
                                y_ps[:, nn * 512:(nn + 1) * 512],
                                R(oall[:, j, :]),
                                R(wo_sb[:, j, nn * 512:(nn + 1) * 512]),
                                start=(j == 0), stop=(j == DCH - 1))
                    y_sb = ypool.tile([128, 1024], F32, tag="ysb",
                                      name=f"ysb{half}{b}")
                    nc.vector.tensor_add(y_sb, y_ps, bo_bc)
                    nc.sync.dma_start(outap[b, half, :, :], y_sb)

            # PE program order: attn(0), attn(1), Y(0), Y(1) -- so the PE
            # never sits behind a collective; each AllToAll overlaps the
            # next chunk of PE work.
            emit_attention(0)
            aout0 = emit_exchange(0)
            emit_attention(1)
            aout1 = emit_exchange(1)
            emit_output(0, aout0)
            emit_output(1, aout1)


_CACHED_NC = None


def _build_program():
    global _CACHED_NC
    if _CACHED_NC is not None:
        return _CACHED_NC
    nc = bacc.Bacc(None, target_bir_lowering=False, debug=False,
                   num_devices=NCORES)
    x = nc.declare_dram_parameter("x", [L, D], F32R, isOutput=False)
    wq = nc.declare_dram_parameter("wq", [D, CPC], F32R, isOutput=False)
    bq = nc.declare_dram_parameter("bq", [CPC, 1], F32, isOutput=False)
    wk = nc.declare_dram_parameter("wk", [D, CPC], F32R, isOutput=False)
    bk = nc.declare_dram_parameter("bk", [CPC, 1], F32, isOutput=False)
    wv = nc.declare_dram_parameter("wv", [D, VA], F32R, isOutput=False)
    bv = nc.declare_dram_parameter("bv", [1, VA], F32, isOutput=False)
    wo = nc.declare_dram_parameter("wo", [D, D], F32R, isOutput=False)
    bo = nc.declare_dram_parameter("bo", [1, D], F32, isOutput=False)
    out = nc.declare_dram_parameter("out", [B, 2, 128, D], F32, isOutput=True)

    with tile.TileContext(nc) as tc:
        _emit(tc, nc, x, wq, bq, wk, bk, wv, bv, wo, bo, out)
    nc.finalize()
    _CACHED_NC = nc
    return nc


def _make_in_maps(X, WQ, bQ, WK, bK, WV, bV, WO, bO):
    X = np.ascontiguousarray(np.asarray(X, np.float32))
    WO = np.ascontiguousarray(np.asarray(WO, np.float32))
    bO = np.asarray(bO, np.float32).reshape(1, D)
    in_maps = []
    for c in range(NCORES):
        b, g = c // 4, c % 4
        cs = slice(CPC * g, CPC * (g + 1))
        wva = np.zeros((D, VA), np.float32)
        bva = np.zeros((1, VA), np.float32)
        for h in range(NH):
            wva[:, h * 65:h * 65 + 64] = WV[:, CPC * g + 64 * h:
                                            CPC * g + 64 * (h + 1)]
            bva[0, h * 65:h * 65 + 64] = bV[CPC * g + 64 * h:
                                            CPC * g + 64 * (h + 1)]
            bva[0, h * 65 + 64] = 1.0
        in_maps.append({
            "x": np.ascontiguousarray(X[b]),
            "wq": np.ascontiguousarray(WQ[:, cs]),
            "bq": np.ascontiguousarray(np.asarray(bQ, np.float32)[cs]
                                       .reshape(CPC, 1)),
            "wk": np.ascontiguousarray(WK[:, cs]),
            "bk": np.ascontiguousarray(np.asarray(bK, np.float32)[cs]
                                       .reshape(CPC, 1)),
            "wv": wva,
            "bv": bva,
            "wo": WO,
            "bo": np.ascontiguousarray(bO),
        })
    return in_maps


def _assemble(results):
    full = np.empty((B, L, D), np.float32)
    for c in range(NCORES):
        o = results[c]["out"]  # [B, 2, 128, D]
        for b in range(B):
            full[b, 128 * c:128 * (c + 1), :] = o[b, 0]
            full[b, 1024 + 128 * c:1024 + 128 * (c + 1), :] = o[b, 1]
    return full


def run(inputs, trace=False):
    nc = _build_program()
    in_maps = _make_in_maps(**inputs)
    res = run_bass_kernel_spmd(nc, in_maps, list(range(NCORES)), trace=trace)
    return _assemble(res.results), res


def kernel(X, WQ, bQ, WK, bK, WV, bV, WO, bO):
    out, _ = run(dict(X=X, WQ=WQ, bQ=bQ, WK=WK, bK=bK, WV=WV, bV=bV,
                      WO=WO, bO=bO))
    return out
